# revision 10
# baseline (speedup 1.0000x reference)
"""Trainium2 Bass kernel for LogicalConsistencyLoss.

loss = W/(R*B) * sum_{b,r} sum_{a,i,c} relu(rel[a,i] - rel[a,c]*rel[i,c])
with rel = sigmoid(logits[b,:,:,r]) masked by the entity_masks outer product
(host folds the mask into the logits as -30).

Distribution: B*R = 8 (batch, relation) matrices -> 8 NeuronCores, one
512x512 matrix per core. Each core returns its scalar partial sum; the
host combines.

Per-core engine split of the N^3 relu work:
 - c in [0,256) (pipeline A, DVE): layout partition=c, free=b. For each a:
   PE computes sel[ai]^T @ relb[ta] = row a of rel replicated across all
   128 partitions (sel[ai] is a constant one-hot-row selector, K=128) into
   PSUM; a custom fused DVE op computes relu(bcast - relT32*C0) with
   C0 = relT32[:, a] and accumulates the free-dim sum into acc_a[:, a]
   in a single 1-elem/cycle fp32 pass.
 - c in [256,512) (pipeline B, PE+ACT): layout partition=a, free=b. Per
   (c, a-tile): PE writes -rel into PSUM ((-I)^T @ relb) then accumulates
   +col_c (x) col_c (K=1 matmul; the row lives in a flat base-partition-
   aligned row store filled via a DRAM round-trip); ScalarE applies
   Relu(scale=-1) with accum_out over a 2-bank [128,1024] PSUM tile
   (two c's at a time).
"""

import sys

if "/opt/trn_rl_repo" not in sys.path:
    sys.path.insert(0, "/opt/trn_rl_repo")

import numpy as np
import ml_dtypes

N = 512
P = 128
NT = N // P          # 4 row tiles
CSPLIT = 256         # c < CSPLIT -> pipeline A (DVE); rest -> pipeline B
NB_ROWS = N - CSPLIT           # rows in the flat store (256)
GROUP = (NB_ROWS + 2) // 3     # rows per base-partition group (86)
TEMPERATURE = 1.0
WEIGHT = 1.0

_CACHE: dict = {}


def _flat_loc(j):
    """Row j (= c - CSPLIT) of the flat store -> (base_partition, elem_offset)."""
    g, q = j // GROUP, j % GROUP
    return 32 * g, q * N


def _get_custom_op():
    """Register (once) the fused DVE op: out = relu(Src1 - Src0*C0),
    accum_out = sum(out)."""
    import concourse.dve_ops as dve_ops
    from concourse.dve_spec import Spec, Src0, Src1, C0, relu, lower
    from concourse.dve_uop import DveOpSpec
    from concourse.dve_table_gen import dve_ver_for
    from operator import add

    name = "LCL_RSUB_MUL_RELU_SUM"
    for o in dve_ops.OPS:
        if o.name == name:
            return o
    spec = Spec(body=relu(Src1 - Src0 * C0), accum=add)
    opc = max(dve_ops._SUB_OPCODE_FOR_NAME.values()) + 1
    assert opc < 0x20
    ver = dve_ver_for("TRN2")
    sha = DveOpSpec(
        name=name, opcode=opc, uops=lower(spec, ver=ver), rd1_en=True
    ).sha(ver)
    op = dve_ops.DveOp(name, spec, subdim=False, uops_sha={ver: sha})
    dve_ops._SUB_OPCODE_FOR_NAME[name] = opc
    dve_ops.OPS.append(op)
    return op


def _build():
    import concourse.bacc as bacc
    import concourse.mybir as mybir
    from concourse.tile import TileContext

    f32 = mybir.dt.float32
    bf16 = mybir.dt.bfloat16
    OP = _get_custom_op()

    nc = bacc.Bacc("TRN2", target_bir_lowering=False)
    x = nc.dram_tensor("x", [N, N], f32, kind="ExternalInput")
    ident32_d = nc.dram_tensor("ident32", [P, P], f32, kind="ExternalInput")
    identbn_d = nc.dram_tensor("identbn", [P, P], bf16, kind="ExternalInput")
    sel_d = nc.dram_tensor("sel", [P, P * P], bf16, kind="ExternalInput")
    ones32_d = nc.dram_tensor("ones32", [P, 1], f32, kind="ExternalInput")
    out_d = nc.dram_tensor("out", [1, 1], f32, kind="ExternalOutput")

    with TileContext(nc) as tc:
        with (
            tc.tile_pool(name="const", bufs=1) as cp,
            tc.tile_pool(name="scr_a", bufs=3) as sa,
            tc.tile_pool(name="dram", bufs=1, space="DRAM") as dp,
            tc.tile_pool(name="pa", bufs=4, space="PSUM") as pa,
            tc.tile_pool(name="pb", bufs=2, space="PSUM") as pb,
        ):
            ident32 = cp.tile([P, P], f32, tag="ident32", name="ident32")
            identbn = cp.tile([P, P], bf16, tag="identbn", name="identbn")
            selt = cp.tile([P, P * P], bf16, tag="selt", name="selt")
            ones32 = cp.tile([P, 1], f32, tag="ones32", name="ones32")
            nc.sync.dma_start(out=ident32, in_=ident32_d[:, :])
            nc.sync.dma_start(out=identbn, in_=identbn_d[:, :])
            nc.sync.dma_start(out=selt, in_=sel_d[:, :])
            nc.sync.dma_start(out=ones32, in_=ones32_d[:, :])

            xt = [cp.tile([P, N], f32, tag=f"xt{t}", name=f"xt{t}")
                  for t in range(NT)]
            rel32 = [cp.tile([P, N], f32, tag=f"rel32{t}", name=f"rel32{t}")
                     for t in range(NT)]
            relb = [cp.tile([P, N], bf16, tag=f"relb{t}", name=f"relb{t}")
                    for t in range(NT)]
            relT32 = [cp.tile([P, N], f32, tag=f"relT32{t}", name=f"relT32{t}")
                      for t in range(NT)]
            tmpb = [cp.tile([P, N], bf16, tag=f"tmpb{t}", name=f"tmpb{t}")
                    for t in range(2)]
            flat = cp.tile([P, GROUP * N], bf16, tag="flat", name="flat")
            acc_a = [cp.tile([P, N], f32, tag=f"acca{t}", name=f"acca{t}")
                     for t in range(CSPLIT // P)]
            acc_b = [cp.tile([P, P], f32, tag=f"accb{t}", name=f"accb{t}")
                     for t in range(NT)]

            for t in range(NT):
                nc.sync.dma_start(out=xt[t], in_=x[t * P:(t + 1) * P, :])
            for t in range(NT):
                nc.scalar.activation(
                    rel32[t], xt[t], mybir.ActivationFunctionType.Sigmoid,
                    scale=1.0 / TEMPERATURE,
                )
                nc.vector.tensor_copy(relb[t], rel32[t])
            # transpose rel32 -> relT32 (16 PE 128x128 blocks)
            for tcol in range(NT):
                for t in range(NT):
                    pt = pa.tile([P, N], f32, tag="pa", name="pa")
                    nc.tensor.transpose(
                        pt[:, :P], rel32[t][:, tcol * P:(tcol + 1) * P], ident32
                    )
                    nc.vector.tensor_copy(
                        relT32[tcol][:, t * P:(t + 1) * P], pt[:, :P]
                    )
            # flat row store: relT rows c in [256,512) at base partitions
            # {0,32,64}, via a DRAM round-trip
            relT_dram = dp.tile([NB_ROWS, N], bf16, name="relT_dram")
            for t in range(2):
                nc.vector.tensor_copy(tmpb[t], relT32[2 + t])
                nc.sync.dma_start(
                    out=relT_dram[t * P:(t + 1) * P, :], in_=tmpb[t]
                )
            for g in range(3):
                r0 = g * GROUP
                nrows = min(GROUP, NB_ROWS - r0)
                nc.sync.dma_start(
                    out=flat[32 * g:32 * g + 1, 0:nrows * N],
                    in_=relT_dram[r0:r0 + nrows, :],
                )

            # ---- main: interleave A iterations (512) and B units (512) ----
            for i in range(N):
                # A: replicated-row broadcast + 2 fused DVE ops (c-tiles 0,1)
                ta, ai = i // P, i % P
                pt = pa.tile([P, N], f32, tag="pa", name="pa")
                nc.tensor.matmul(
                    pt, selt[:, ai * P:(ai + 1) * P], relb[ta],
                    start=True, stop=True,
                )
                for tcol in range(CSPLIT // P):
                    so = sa.tile([P, N], bf16, tag="scr_a", name="scr_a")
                    nc.vector._custom_dve(
                        OP,
                        out=so,
                        in0=relT32[tcol],
                        in1=pt,
                        s0=relT32[tcol][:, i:i + 1],
                        accum_out=acc_a[tcol][:, i:i + 1],
                    )
                # B: one (c-pair, a-tile) unit; PSUM = col (x) col - rel
                j, tb = i // 4, i % 4
                pbt = pb.tile([P, 2 * N], f32, tag="pb", name="pb")
                for k in range(2):
                    c = CSPLIT + 2 * j + k
                    bp, off = _flat_loc(c - CSPLIT)
                    half = pbt[:, k * N:(k + 1) * N]
                    nc.tensor.matmul(half, identbn, relb[tb],
                                     start=True, stop=False)
                    nc.tensor.matmul(
                        half,
                        flat[bp:bp + 1, off + tb * P:off + (tb + 1) * P],
                        flat[bp:bp + 1, off:off + N],
                        start=False, stop=True,
                    )
                nc.scalar.activation(
                    pbt, pbt, mybir.ActivationFunctionType.Relu,
                    scale=-1.0,
                    accum_out=acc_b[tb][:, j:j + 1],
                )

            # ---- final reduction ----
            parts = []
            for t in range(CSPLIT // P):
                r = cp.tile([P, 1], f32, tag=f"ra{t}", name=f"ra{t}")
                nc.vector.tensor_reduce(
                    r, acc_a[t], axis=mybir.AxisListType.X, op=mybir.AluOpType.add
                )
                parts.append(r)
            for t in range(NT):
                r = cp.tile([P, 1], f32, tag=f"rb{t}", name=f"rb{t}")
                nc.vector.tensor_reduce(
                    r, acc_b[t], axis=mybir.AxisListType.X, op=mybir.AluOpType.add
                )
                parts.append(r)
            tot = parts[0]
            for r in parts[1:]:
                nc.vector.tensor_add(tot, tot, r)
            pt = pa.tile([P, N], f32, tag="pa", name="pa")
            nc.tensor.matmul(pt[0:1, 0:1], tot, ones32, start=True, stop=True)
            out_sb = cp.tile([1, 1], f32, tag="out_sb", name="out_sb")
            nc.vector.tensor_copy(out_sb, pt[0:1, 0:1])
            nc.sync.dma_start(out=out_d[:, :], in_=out_sb)

    nc.compile()
    return nc


def _get_nc():
    if "nc" not in _CACHE:
        _CACHE["nc"] = _build()
    return _CACHE["nc"]


def _consts():
    if "consts" not in _CACHE:
        sel = np.zeros((P, P, P), dtype=ml_dtypes.bfloat16)
        for i in range(P):
            sel[i, i, :] = 1  # sel layout on host: [k, ai, m]
        sel = np.ascontiguousarray(np.transpose(sel, (1, 0, 2)))
        _CACHE["consts"] = {
            "ident32": np.eye(P, dtype=np.float32),
            "identbn": (-np.eye(P)).astype(ml_dtypes.bfloat16),
            "sel": sel.reshape(P, P * P),
            "ones32": np.ones((P, 1), dtype=np.float32),
        }
    return _CACHE["consts"]


def kernel(relation_logits: np.ndarray, entity_masks: np.ndarray) -> np.ndarray:
    from concourse.bass_utils import run_bass_kernel_spmd

    B, n, _, R = relation_logits.shape
    assert (n, B * R) == (N, 8)
    x = np.ascontiguousarray(
        np.transpose(np.asarray(relation_logits, dtype=np.float32), (0, 3, 1, 2))
    ).reshape(B * R, N, N)
    m = np.asarray(entity_masks) > 0
    for b in range(B):
        if not m[b].all():
            keep = np.outer(m[b], m[b])
            x[b * R:(b + 1) * R][:, ~keep] = -30.0

    consts = _consts()
    in_maps = [{"x": x[i], **consts} for i in range(8)]
    res = run_bass_kernel_spmd(_get_nc(), in_maps, list(range(8)))
    total = float(sum(float(r["out"][0, 0]) for r in res.results))
    return np.float32(WEIGHT * total / (R * B))
